# revision 1
# baseline (speedup 1.0000x reference)
"""Fused attention block (nn_Attention_27865747817251) on 8 trn2 NeuronCores.

v2: fp8e4 DoubleRow (0.5 cy/row, K=256/instr) for the QKV projection and the
output projection via 3-term hi/lo bilinear splits; attention A@V reformulated
with pT as the stationary operand and a ones-augmented V as moving, so the
softmax denominator falls out of the same matmul (no separate ones-matmul) and
the output lands [token-partition, dim-free] where normalization is a cheap
per-partition-scalar DVE multiply.

Numeric scheme (validated in numpy; measured HW rel err 4.8e-3 vs 2e-2 gate):
  y    = (x_hi@(32w)_hi + (x-x_hi)_fp8@(32w)_hi + x_hi@(32w-wh)) / 32, bf16
  S^T  = K@Q^T in bf16; pT = exp(S^T/sqrt(HD)) bf16
  ob   = 64*out = (pT.T@[V | 1/64]) row-normalized, bf16; hi=fp8(ob^T),
         lo=fp8(ob^T-hi)
  fin  = hi@(32wo)_hi + lo@(32wo)_hi + hi@(32wo-woh)  [= 2048*out@wo],
         host divides by 2048 and adds b_out.

Sharding: launch A core k=(batch k//4, col-group k%4 of y^T); launch B core
k=(batch, head-group of 4): attention + row-parallel out-proj partials summed
on the host.
"""

from contextlib import ExitStack

import numpy as np
import ml_dtypes

import concourse.bass as bass
from concourse import bacc
import concourse.mybir as mybir
import concourse.tile as tile
from concourse import masks
from concourse.bass_utils import run_bass_kernel_spmd

B, L, D = 2, 2048, 2048
NH, HD = 16, 128
D3 = 3 * D
NCHUNK = D3 // 128              # 48 column chunks of y
CPC = NCHUNK // 4               # 12 chunks per core (launch A)
KT = D // 128                   # 16 k-subtiles of 128
SCALE = 1.0 / float(np.sqrt(HD))

BF = mybir.dt.bfloat16
F8 = mybir.dt.float8e4
NPBF = ml_dtypes.bfloat16
NPF8 = ml_dtypes.float8_e4m3
DR = mybir.MatmulPerfMode.DoubleRow

_CACHE = {}


def _warmup(nc, sbuf_pool, psum_pool, tag, n=24):
    """Keep the PE busy from t~0 so the p-state ramp (0.65->2.4GHz over 3us)
    completes during the initial DMA fill instead of during real matmuls.
    Narrow tiles: the goal is to SPAN ~3us of wall clock with minimal PE
    cycles burned.  Borrows the psum pool's existing tag (no extra banks)."""
    a = sbuf_pool.tile([128, 128], BF, tag="wua")
    nc.vector.memset(a[:], 0.0)
    for _ in range(n):
        ps = psum_pool.tile([128, 128], mybir.dt.float32, tag=tag)
        nc.tensor.matmul(ps[:], a[:], a[:], start=True, stop=True)
    return a


def _build_launch_a(reps=1):
    """Core k=(b, cg): y^T chunks [CPC, 128, L] bf16 via 3-term fp8 DoubleRow."""
    nc = bacc.Bacc()
    xh = nc.dram_tensor("xh", [128, KT, L], F8, kind="ExternalInput")
    xl = nc.dram_tensor("xl", [128, KT, L], F8, kind="ExternalInput")
    wh = nc.dram_tensor("wh", [128, CPC, KT, 128], F8, kind="ExternalInput")
    wl = nc.dram_tensor("wl", [128, CPC, KT, 128], F8, kind="ExternalInput")
    yt = nc.dram_tensor("yt", [CPC, 128, L], BF, kind="ExternalOutput")

    with tile.TileContext(nc) as tc, ExitStack() as ctx:
        singles = ctx.enter_context(tc.tile_pool(name="singles", bufs=1))
        # out tiles cover the whole launch: the serialized DMA device drains
        # input loads first, so output DMAs back up ~50us — without full
        # buffering that WAR-chains back into the matmul stream.
        outs = ctx.enter_context(tc.tile_pool(name="outs", bufs=48))
        psum = ctx.enter_context(tc.tile_pool(name="psum", bufs=8, space="PSUM"))

        for _rep in range(reps):
            _warmup(nc, singles, psum, "p")
            xh_sb = singles.tile([128, KT, L], F8, tag="xh")
            xl_sb = singles.tile([128, KT, L], F8, tag="xl")
            wh_sb = singles.tile([128, CPC, KT, 128], F8, tag="wh")
            wl_sb = singles.tile([128, CPC, KT, 128], F8, tag="wl")

            # deadline-ordered loads: first quarter-strip + w chunk 0
            # first (PE can start ~3us in), then the remaining w chunks ahead
            # of the rp0 cb sweep, then the later x strips.
            def w_chunks(cbs):
                for cb in cbs:
                    nc.sync.dma_start(wh_sb[:, cb], wh[:, cb])
                    nc.sync.dma_start(wl_sb[:, cb], wl[:, cb])

            # full 512-col strips only: narrower slices have <512B
            # contiguous runs (fp8) and pay the 2x small-element DMA penalty.
            nc.sync.dma_start(xh_sb[:, :, 0:512], xh[:, :, 0:512])
            w_chunks([0])
            nc.sync.dma_start(xl_sb[:, :, 0:512], xl[:, :, 0:512])
            w_chunks(range(1, 6))
            nc.sync.dma_start(xh_sb[:, :, 512:1024], xh[:, :, 512:1024])
            nc.sync.dma_start(xl_sb[:, :, 512:1024], xl[:, :, 512:1024])
            w_chunks(range(6, CPC))
            for st in range(2, 4):
                sl = slice(st * 512, (st + 1) * 512)
                nc.sync.dma_start(xh_sb[:, :, sl], xh[:, :, sl])
                nc.sync.dma_start(xl_sb[:, :, sl], xl[:, :, sl])
            # rb-pair-major: 512 l-cols per (rp, cb) out tile.  The final
            # tiles stream their halves out eagerly so the closing DMA chain
            # overlaps the last matmuls.
            for rp in range(L // 512):
                for cb in range(CPC):
                    eager = (rp == L // 512 - 1 and cb >= CPC - 2)
                    ot = outs.tile([128, 512], BF, tag="o")
                    for sub in range(2):
                        r0 = rp * 512 + sub * 256
                        pt = psum.tile([128, 256], mybir.dt.float32, tag="p")
                        first = True
                        for term, (wsb, xsb) in enumerate(
                            ((wh_sb, xh_sb), (wl_sb, xh_sb), (wh_sb, xl_sb))
                        ):
                            for kp in range(KT // 2):
                                nc.tensor.matmul(
                                    pt[:],
                                    wsb[:, cb, 2 * kp:2 * kp + 2, :],
                                    xsb[:, 2 * kp:2 * kp + 2, r0:r0 + 256],
                                    start=first,
                                    stop=(term == 2 and kp == KT // 2 - 1),
                                    perf_mode=DR,
                                )
                                first = False
                        osl = ot[:, sub * 256:(sub + 1) * 256]
                        if sub == 0:
                            nc.scalar.activation(
                                osl, pt[:], mybir.ActivationFunctionType.Copy,
                                scale=1.0 / 32.0)
                        else:
                            nc.vector.tensor_scalar_mul(osl, pt[:], 1.0 / 32.0)
                        if eager:
                            nc.sync.dma_start(
                                yt[cb, :, r0:r0 + 256], osl)
                    if not eager:
                        nc.sync.dma_start(
                            yt[cb, :, rp * 512:(rp + 1) * 512], ot[:])
    nc.compile()
    return nc


def _build_launch_b(reps=1):
    """Core (b,g): attention for 4 heads + fp8 row-parallel out-proj partial."""
    HPC = 4
    nc = bacc.Bacc()
    qt = nc.dram_tensor("qt", [128, HPC, L], BF, kind="ExternalInput")
    kt_ = nc.dram_tensor("kt", [128, HPC, L], BF, kind="ExternalInput")
    va = nc.dram_tensor("va", [128, HPC, 16 * 129], BF, kind="ExternalInput")
    who = nc.dram_tensor("who", [128, HPC, D], F8, kind="ExternalInput")
    whol = nc.dram_tensor("whol", [128, HPC, D], F8, kind="ExternalInput")
    fp = nc.dram_tensor("fp", [L, D], BF, kind="ExternalOutput")

    F32 = mybir.dt.float32
    with tile.TileContext(nc) as tc, ExitStack() as ctx:
        singles = ctx.enter_context(tc.tile_pool(name="singles", bufs=1))
        pts = ctx.enter_context(tc.tile_pool(name="pts", bufs=32))
        small = ctx.enter_context(tc.tile_pool(name="small", bufs=8))
        fouts = ctx.enter_context(tc.tile_pool(name="fouts", bufs=8))
        pss = ctx.enter_context(tc.tile_pool(name="pss", bufs=2, space="PSUM"))
        pav = ctx.enter_context(tc.tile_pool(name="pav", bufs=2, space="PSUM"))
        pfin = ctx.enter_context(tc.tile_pool(name="pfin", bufs=2, space="PSUM"))

        for _rep in range(reps):
            wua = _warmup(nc, singles, pfin, "f")
            qt_sb = singles.tile([128, HPC, L], BF, tag="qt")
            kt_sb = singles.tile([128, HPC, L], BF, tag="kt")
            va_sb = singles.tile([128, HPC, 16 * 129], BF, tag="va")
            who_sb = singles.tile([128, HPC, D], F8, tag="who")
            whol_sb = singles.tile([128, HPC, D], F8, tag="whol")
            oth = singles.tile([128, HPC, L], F8, tag="oth")
            otl = singles.tile([128, HPC, L], F8, tag="otl")
            ident = singles.tile([128, 128], BF, tag="ident")
            masks.make_identity(nc, ident[:])
            nc.sync.dma_start(qt_sb[:, 0, 0:1024], qt[:, 0, 0:1024])
            nc.sync.dma_start(kt_sb[:, 0, 0:1024], kt_[:, 0, 0:1024])
            nc.sync.dma_start(qt_sb[:, 0, 1024:], qt[:, 0, 1024:])
            nc.sync.dma_start(kt_sb[:, 0, 1024:], kt_[:, 0, 1024:])
            nc.sync.dma_start(va_sb[:, 0, :], va[:, 0, :])
            for hh in range(1, HPC):
                nc.sync.dma_start(qt_sb[:, hh, :], qt[:, hh, :])
                nc.sync.dma_start(kt_sb[:, hh, :], kt_[:, hh, :])
                nc.sync.dma_start(va_sb[:, hh, :], va[:, hh, :])
            nc.sync.dma_start(who_sb[:], who[:])
            nc.sync.dma_start(whol_sb[:], whol[:])

            parts = {}

            def outproj_group(icg, ccp, act_copy=False, hps=(0, 2)):
                # hps=(0,): heads 0-1 only -> bf16 partial stashed in SBUF
                # hps=(2,): heads 2-3      -> psum + stashed partial -> out
                isl = slice(icg * 128, (icg + 1) * 128)
                pf = pfin.tile([128, 512], F32, tag="f")
                for sub in range(2):
                    c0 = ccp * 512 + sub * 256
                    csl = slice(c0, c0 + 256)
                    first = True
                    for wsb, osb in ((who_sb, oth), (who_sb, otl),
                                     (whol_sb, oth)):
                        for hp in hps:
                            nc.tensor.matmul(
                                pf[:, sub * 256:(sub + 1) * 256],
                                osb[:, hp:hp + 2, isl],
                                wsb[:, hp:hp + 2, csl],
                                start=first,
                                stop=(wsb is whol_sb and hp == hps[-1]),
                                perf_mode=DR,
                            )
                            first = False
                if hps == (0,):
                    pp = fouts.tile([128, 512], BF, tag="pp", bufs=34)
                    nc.vector.tensor_copy(pp[:], pf[:])
                    parts[(icg, ccp)] = pp
                    return
                fo = fouts.tile([128, 512], BF, tag="fo")
                pp = parts.pop((icg, ccp), None)
                if pp is not None:
                    nc.vector.tensor_add(fo[:], pf[:], pp[:])
                elif act_copy:
                    nc.scalar.copy(fo[:], pf[:])
                else:
                    nc.vector.tensor_copy(fo[:], pf[:])
                nc.sync.dma_start(
                    fp[isl, ccp * 512:(ccp + 1) * 512], fo[:])

            blocks = [(ih, hh) for ih in range(2) for hh in range(HPC)]
            pts_of = {}
            def issue_scores_jb(bi, jb):
                ih, hh = blocks[bi]
                i0 = ih * 1024
                ps_s = pss.tile([128, 1024], F32, tag="s")
                jsl = slice(jb * 128, (jb + 1) * 128)
                for half in range(2):
                    nc.tensor.matmul(
                        ps_s[:, half * 512:(half + 1) * 512],
                        qt_sb[:, hh, jsl],
                        kt_sb[:, hh, i0 + half * 512:i0 + (half + 1) * 512],
                        start=True, stop=True,
                    )
                pT = pts.tile([128, 1024], BF, tag="pT")
                nc.scalar.activation(
                    pT[:], ps_s[:],
                    mybir.ActivationFunctionType.Exp, scale=SCALE)
                pts_of.setdefault(bi, []).append(pT)

            # software pipeline: block X+1's scores/exp are issued two jb at a
            # time inside block X's AV loop, so the Act engine's exp stream
            # never starves and the scores-psum WAR never stalls a burst.
            # Out-proj chunks drain into the same stream two AV steps after
            # their head-3 planes complete.
            pending = []
            seq = [0]

            def drain(budget=1, lag=2):
                n = 0
                while budget and pending and pending[0][3] <= seq[0] - lag:
                    icg, ccp, hps, _ = pending.pop(0)
                    outproj_group(icg, ccp, hps=hps)
                    budget -= 1
                    n += 1
                return n

            def keep_warm(n=8):
                # PE idle would reset the p-state ramp (2x clock penalty for
                # the next 3us); burn a few cheap matmuls instead.
                for _ in range(n):
                    ps = pfin.tile([128, 128], F32, tag="f")
                    nc.tensor.matmul(ps[:], wua[:], wua[:],
                                     start=True, stop=True)

            for jb in range(16):
                issue_scores_jb(0, jb)
            for bi in range(8):
                ih, hh = blocks[bi]
                ptiles = pts_of.pop(bi)
                for ic in range(8):
                    po = pav.tile([128, 512], F32, tag="av")
                    for jb in range(16):
                        nc.tensor.matmul(
                            po[:, 0:129],
                            ptiles[jb][:, ic * 128:(ic + 1) * 128],
                            va_sb[:, hh, jb * 129:(jb + 1) * 129],
                            start=(jb == 0), stop=(jb == 15),
                        )
                    recip = small.tile([128, 1], F32, tag="r")
                    nc.vector.reciprocal(recip[:], po[:, 128:129])
                    ob = small.tile([128, 128], BF, tag="ob")
                    nc.vector.tensor_scalar_mul(ob[:], po[:, 0:128], recip[:])
                    psT = pfin.tile([128, 128], BF, tag="f")
                    nc.tensor.transpose(psT[:], ob[:], ident[:])
                    obT = small.tile([128, 128], BF, tag="obT")
                    nc.vector.tensor_copy(obT[:], psT[:])
                    icg = ih * 8 + ic
                    osl = slice(icg * 128, (icg + 1) * 128)
                    nc.gpsimd.tensor_copy(oth[:, hh, osl], obT[:])
                    nc.gpsimd.tensor_sub(otl[:, hh, osl], obT[:],
                                         oth[:, hh, osl])
                    if ih == 0 and hh == 1:
                        # heads 0-1 planes complete for this ih0 i-chunk:
                        # the hp01 half-outproj can fill ih0's Act-bound
                        # blocks 2-3, where the PE otherwise underruns.
                        for ccp in range(4):
                            pending.append((icg, ccp, (0,), seq[0]))
                    if hh == HPC - 1:
                        for ccp in range(4):
                            pending.append(
                                (icg, ccp, (2,) if ih == 0 else (0, 2),
                                 seq[0]))
                    seq[0] += 1
                    drain(*((4, 1) if bi == 7 else (2, 2) if bi in (2, 3) else (1, 3)))
                    if bi + 1 < 8:
                        issue_scores_jb(bi + 1, 2 * ic)
                        issue_scores_jb(bi + 1, 2 * ic + 1)
            tail_i = [0]
            while pending:
                icg, ccp, hps, _ = pending.pop(0)
                outproj_group(icg, ccp, tail_i[0] % 2 == 0, hps=hps)
                tail_i[0] += 1
    nc.compile()
    return nc


def _get(name, reps=1):
    key = (name, reps)
    if key not in _CACHE:
        _CACHE[key] = (_build_launch_a(reps) if name == "a"
                       else _build_launch_b(reps))
    return _CACHE[key]


def _prep_a(x, w_qkv):
    in_a = []
    xplanes = {}
    for b in range(B):
        xt = np.ascontiguousarray(
            x[b].T.reshape(KT, 128, L).transpose(1, 0, 2))
        xh = xt.astype(NPF8)
        xl = (xt - xh.astype(np.float32)).astype(NPF8)
        xplanes[b] = (xh, xl)
    for k in range(8):
        b, cg = k // 4, k % 4
        wsl = w_qkv[:, cg * CPC * 128:(cg + 1) * CPC * 128]
        # [D, CPC*128] -> [128(d in kt), CPC, KT, 128(col)]
        w4 = np.ascontiguousarray(
            wsl.reshape(KT, 128, CPC, 128).transpose(1, 2, 0, 3))
        wh = (w4 * 32.0).astype(NPF8)
        wl = (w4 * 32.0 - wh.astype(np.float32)).astype(NPF8)
        xh, xl = xplanes[b]
        in_a.append({"xh": xh, "xl": xl, "wh": wh, "wl": wl})
    return in_a


def _prep_b(ya_list, b_qkv, w_out):
    sec = L * HD
    in_b = []
    for b in range(B):
        yb = np.concatenate([ya_list[b * 4 + cg] for cg in range(4)], axis=0)
        if b_qkv.any():
            yb = (yb.astype(np.float32)
                  + b_qkv.reshape(NCHUNK, 128)[:, :, None]).astype(NPBF)
        flat = np.ascontiguousarray(yb.transpose(2, 0, 1)).reshape(-1)
        for g in range(4):
            qts, kts, vas = [], [], []
            for hh in range(4):
                h = 4 * g + hh
                qh = flat[h * sec:(h + 1) * sec].reshape(L, HD)
                kh = flat[(NH + h) * sec:(NH + h + 1) * sec].reshape(L, HD)
                vh = flat[(2 * NH + h) * sec:(2 * NH + h + 1) * sec].reshape(L, HD)
                qts.append(qh.T)
                kts.append(kh.T)
                # vaug [j=128, jb, 129]: v rows jb*128+j, plus 1/64 col
                vv = np.empty((16, 128, 129), dtype=NPBF)
                vv[:, :, :128] = vh.reshape(16, 128, HD)
                vv[:, :, 128] = NPBF(1.0 / 64.0)
                vas.append(vv.transpose(1, 0, 2).reshape(128, 16 * 129))
            wsl = w_out[g * 512:(g + 1) * 512, :]
            w3 = np.ascontiguousarray(
                wsl.reshape(4, 128, D).transpose(1, 0, 2))
            who = (w3 * 32.0).astype(NPF8)
            whol = (w3 * 32.0 - who.astype(np.float32)).astype(NPF8)
            in_b.append({
                "qt": np.ascontiguousarray(np.stack(qts, axis=1)),
                "kt": np.ascontiguousarray(np.stack(kts, axis=1)),
                "va": np.ascontiguousarray(np.stack(vas, axis=1)),
                "who": who, "whol": whol,
            })
    return in_b


def kernel(x, w_qkv, b_qkv, w_out, b_out, _timing=None):
    x = np.asarray(x, dtype=np.float32)
    w_qkv = np.asarray(w_qkv, dtype=np.float32)
    b_qkv = np.asarray(b_qkv, dtype=np.float32)
    w_out = np.asarray(w_out, dtype=np.float32)
    b_out = np.asarray(b_out, dtype=np.float32)
    cores = list(range(8))

    in_a = _prep_a(x, w_qkv)
    res_a = run_bass_kernel_spmd(_get("a"), in_a, cores)
    ya = [np.asarray(res_a.results[k]["yt"]) for k in range(8)]

    in_b = _prep_b(ya, b_qkv, w_out)
    res_b = run_bass_kernel_spmd(_get("b"), in_b, cores)

    out = np.empty((B, L, D), dtype=np.float32)
    for b in range(B):
        acc = np.zeros((L, D), dtype=np.float32)
        for g in range(4):
            acc += np.asarray(res_b.results[b * 4 + g]["fp"]).astype(np.float32)
        out[b] = acc * (1.0 / 2048.0) + b_out[None, :]
    return out

